# revision 32
# baseline (speedup 1.0000x reference)
"""Trainium2 Bass kernel for nn_KernelEncoderLayer (gnn_message_passing).

v7: both BN reductions run as 3-round XOR-butterfly all-reduces on
remote_dma_broadcast. The mandatory launch-sync collective (a NEFF needs
one full-group CC for coordinated dispatch) is a dummy whose PSEUDO_TRIGGER
sits between the two butterflies on the gpsimd queue: fired at ~40us it
hits the fixed ~64us bootstrap wall (not trigger+19), its completion
re-synchronizes the cores right before butterfly-B, and
tile_critical(no_gpsimd_drain=True) lets the BN0 chain + h compute proceed
on the other engines while gpsimd parks on the CC wait.

Self-contained: hardcodes B=4, N=1024, K=9, C=32, CM=128, 8 cores.
"""

import numpy as np
import ml_dtypes

import concourse.bass as bass
import concourse.bacc as bacc
import concourse.mybir as mybir
import concourse.tile as tile
from concourse import masks
from concourse.bass_utils import run_bass_kernel_spmd

F32 = mybir.dt.float32
BF16 = mybir.dt.bfloat16
AF = mybir.ActivationFunctionType
ALU = mybir.AluOpType
AX = mybir.AxisListType

NB, N, K, C, CM = 4, 1024, 9, 32, 128
NCORES = 4
EPS = 1e-5
SLOPE = 0.01
NT = NB * N
KC = K * C  # 288


def _build_module():
    nc = bacc.Bacc("TRN2", target_bir_lowering=False, debug=False,
                   num_devices=NCORES)

    def din(name, shape, dt=F32):
        return nc.dram_tensor(name, list(shape), dt, kind="ExternalInput").ap()

    packp_d = din("packp", (8, 2048), BF16)   # pn8 | pi8 hi/lo coord rows
    packw_d = din("packw", (C, 2464), BF16)   # wTb | wtob | w1b16 | cwall
    packf_d = din("packf", (128, 4608), BF16)  # bexp | aexp (c-replicated)
    packs_d = din("packs", (C + 1, 1156))      # w1e | g0,be0,b2 | wto
    p128_d = din("p128", (128, 44))           # negsqn(8)|w2(32)|g1|be1|pad

    out_d = nc.dram_tensor("out", [C, 1024], F32, kind="ExternalOutput").ap()

    rsemB = [nc.alloc_semaphore(f"arB_r{r}") for r in range(3)]
    lsem = nc.alloc_semaphore("ar_lsem")
    psemB = nc.alloc_semaphore("ar_psemB")
    vsemB = nc.alloc_semaphore("ar_vsemB")

    DMY_GROUPS = [list(range(NCORES))]

    with tile.TileContext(nc) as tc:
        with (
            tc.tile_pool(name="const", bufs=1) as pc,
            tc.tile_pool(name="big", bufs=1) as pb,
            tc.tile_pool(name="work", bufs=3) as pw,
            tc.tile_pool(name="dram", bufs=1, space="DRAM") as pd,
        ):
            agS_in = pd.tile([C, 8], F32, tag="agS_in")
            agS_out = pd.tile([C * NCORES, 8], F32, tag="agS_out")

            def load(name, ap, shape, dt=F32, pool=pc):
                t = pool.tile(list(shape), dt, tag=name, name=name)
                nc.sync.dma_start(out=t[:], in_=ap)
                return t

            p128 = load("p128", p128_d, (128, 44))
            packp = load("packp", packp_d, (8, 2048), BF16)
            packw = load("packw", packw_d, (C, 2464), BF16)
            packf = load("packf", packf_d, (128, 4608), BF16, pool=pb)
            packs = load("packs", packs_d, (C + 1, 1156))

            pn8 = packp[:, 0:N]
            pi8 = packp[:, N:N + 1024]
            wTb = packw[:, 0:1024]
            wtob = packw[:, 1024:2048]
            w1b16 = packw[:, 2048:2176]
            cwall = packw[:, 2176:2464]
            bexp = packf[:, 0:8 * KC]
            aexp = packf[:, 8 * KC:16 * KC]
            w1e = packs[:, 0:CM]
            g0c = packs[0:C, 128:129]
            be0c = packs[0:C, 129:130]
            b2c = packs[0:C, 130:131]
            wto = packs[0:C, 131:1155]

            negsqn = p128[:, 0:8]
            w2 = p128[:, 8:40]
            g1c = p128[:, 40:41]
            be1c = p128[:, 41:42]

            ident = pc.tile([128, 128], F32, tag="ident")
            masks.make_identity(nc, ident[:])
            warm0 = pc.tile([128, 1], F32, tag="warm0")
            nc.scalar.activation(warm0[:], ident[:, 0:1], AF.Exp)

            ag_y = pb.tile([C, 1024], BF16, tag="ag_y")
            ysum_p = pc.tile([C, 8], F32, tag="ysum")
            ysq_p = pc.tile([C, 8], F32, tag="ysq")

            stat_sb = pc.tile([C, 8], F32, tag="stat_sb")
            accB = [pb.tile([128, 8], F32, tag=f"accB{r}", name=f"accB{r}")
                    for r in range(4)]
            rxB = [pb.tile([128, 8], F32, tag=f"rxB{r}", name=f"rxB{r}")
                   for r in range(3)]

            def butterfly(acc, rx, rsems, psem, vsem):
                for r, d in enumerate([1, 2]):
                    if r > 0:
                        nc.gpsimd.wait_ge(vsem, r)
                    rdests = [None] * 8
                    rdests[4 if d & 4 else 0] = (0, d)
                    nc.gpsimd.remote_dma_broadcast(
                        out_ap=rx[r][:], in_ap=acc[r][:],
                        remote_sem=rsems[r], local_sem=lsem,
                        rdests=rdests).then_inc(psem, 1)
                    nc.gpsimd.wait_ge(psem, r + 1)
                    nc.gpsimd.trigger_dma(count=1)
                    nc.vector.wait_ge(rsems[r], 2)
                    nc.vector.tensor_tensor(
                        acc[r + 1][:], acc[r][:], rx[r][:], op=ALU.add)
                    if r < 2:
                        nc.vector.sem_inc(vsem, 1)

            with (
                tc.tile_pool(name="psG", bufs=3, space="PSUM") as psG,
                tc.tile_pool(name="psB", bufs=2, space="PSUM") as psB,
                tc.tile_pool(name="psD", bufs=2, space="PSUM") as psD,
                tc.tile_pool(name="psC", bufs=1, space="PSUM") as psC,
            ):
                g_sb = [pb.tile([128, 1024], BF16, tag=f"g{j}", name=f"g{j}")
                        for j in range(8)]
                cw_sb = [pb.tile([128, KC], BF16, tag=f"cw{j}", name=f"cw{j}")
                         for j in range(8)]
                for j in range(8):
                    for ih in range(2):
                        psg = psG.tile([128, 512], F32, tag="g")
                        nc.tensor.matmul(psg[:],
                                         lhsT=pn8[:, j * 128:(j + 1) * 128],
                                         rhs=pi8[:, ih * 512:ih * 512 + 512],
                                         start=True, stop=True)
                        nc.scalar.activation(
                            g_sb[j][:, ih * 512:ih * 512 + 512], psg[:],
                            AF.Exp, bias=negsqn[:, j:j + 1], scale=1.0)
                    psb = psB.tile([128, KC], F32, tag="b")
                    nc.tensor.matmul(psb[:], lhsT=wTb[:, j * 128:(j + 1) * 128],
                                     rhs=cwall, start=True, stop=True)
                    nc.vector.tensor_tensor(
                        cw_sb[j][:], psb[:],
                        bexp[:, j * KC:(j + 1) * KC], op=ALU.mult)

                for tp in range(4):
                    pys = [psD.tile([128, KC], F32, tag="py",
                                    name=f"py{tp}_{ti}")
                           for ti in range(2)]
                    for j in range(8):
                        for ti in range(2):
                            t = tp * 2 + ti
                            nc.tensor.matmul(
                                pys[ti][:],
                                lhsT=g_sb[j][:, t * 128:(t + 1) * 128],
                                rhs=cw_sb[j][:],
                                start=(j == 0), stop=(j == 7))
                    for ti in range(2):
                        t = tp * 2 + ti
                        ya = pw.tile([128, KC], F32, tag="ya")
                        nc.vector.tensor_tensor(
                            ya[:], pys[ti][:], aexp[:, t * KC:(t + 1) * KC],
                            op=ALU.mult)
                        y_t = pw.tile([128, C], F32, tag="yt")
                        nc.vector.tensor_reduce(
                            y_t[:], ya[:].rearrange("p (k c) -> p c k", k=K),
                            axis=AX.X, op=ALU.add)
                        y_l = pw.tile([128, C], F32, tag="yl")
                        nc.vector.scalar_tensor_tensor(
                            y_l[:], y_t[:], SLOPE, y_t[:],
                            op0=ALU.mult, op1=ALU.max)
                        ptr = psC.tile([C, 128], F32, tag="tr")
                        nc.tensor.transpose(ptr[:], y_l[:], ident[:])
                        nc.vector.tensor_scalar(
                            ag_y[:, t * 128:(t + 1) * 128], ptr[:], 0.0, 0.0,
                            op0=ALU.add, op1=ALU.add,
                            accum_out=ysum_p[:, t:t + 1])
                        agt = ag_y[:, t * 128:(t + 1) * 128]
                        sq = pw.tile([C, 128], BF16, tag="sq")
                        nc.vector.scalar_tensor_tensor(
                            sq[:], agt, 1.0, agt, op0=ALU.mult, op1=ALU.mult,
                            accum_out=ysq_p[:, t:t + 1])

                nc.vector.tensor_reduce(stat_sb[:, 0:1], ysum_p[:],
                                        axis=AX.X, op=ALU.add)
                nc.vector.tensor_reduce(stat_sb[:, 1:2], ysq_p[:],
                                        axis=AX.X, op=ALU.add)

            # ---- BN0 stats via the launch-sync collective (the one CC the
            # NEFF needs for coordinated dispatch): triggered ~25us, its
            # mesh is wall-bound (~64-71us) and synchronizes the cores for
            # the post-mesh BN1 butterfly.
            nc.sync.dma_start(out=agS_in[:], in_=stat_sb[:])
            nc.gpsimd.collective_compute(
                "AllGather", ALU.bypass,
                replica_groups=DMY_GROUPS,
                ins=[agS_in[:].opt()], outs=[agS_out[:].opt()])

            with (
                tc.tile_pool(name="psR", bufs=2, space="PSUM") as psR,
                tc.tile_pool(name="psT", bufs=1, space="PSUM") as psT,
            ):
                psoL = psR.tile([128, 512], F32, tag="hr", name="psoL")
                psoR = psR.tile([128, 512], F32, tag="hr", name="psoR")
                with tc.high_priority():
                    nc.tensor.matmul(psoL[:], lhsT=w1b16, rhs=wtob[:, 0:512],
                                     start=True, stop=False)
                    nc.tensor.matmul(psoR[:], lhsT=w1b16,
                                     rhs=wtob[:, 512:1024],
                                     start=True, stop=False)
                    warm = pw.tile([128, 1], F32, tag="warm")
                    nc.scalar.activation(warm[:], p128[:, 42:43], AF.Sqrt)

                statj = pc.tile([C, 8], F32, tag="statj")
                agvS = agS_out[:].rearrange("(jj p) n -> p jj n", jj=NCORES)
                nc.sync.dma_start(
                    out=statj[:].rearrange("p (jj s) -> p jj s", s=2),
                    in_=agvS[:, :, 0:2])
                statw = statj[:].rearrange("p (jj s) -> p s jj", s=2)
                tot = pc.tile([C, 2], F32, tag="tot")
                nc.vector.tensor_reduce(tot[:, 0:1], statw[:, 0:1, :],
                                        axis=AX.X, op=ALU.add)
                nc.vector.tensor_reduce(tot[:, 1:2], statw[:, 1:2, :],
                                        axis=AX.X, op=ALU.add)
                mom = pc.tile([C, 2], F32, tag="mom")
                nc.vector.tensor_scalar_mul(mom[:], tot[:], 1.0 / NT)
                var0 = pc.tile([C, 1], F32, tag="var0")
                nc.vector.tensor_tensor(var0[:], mom[:, 0:1], mom[:, 0:1],
                                        op=ALU.mult)
                nc.vector.scalar_tensor_tensor(
                    var0[:], mom[:, 1:2], EPS, var0[:],
                    op0=ALU.add, op1=ALU.subtract)

                def rsqrt(dst, src, p, pref):
                    st = pc.tile([p, 1], F32, tag=pref + "s")
                    nc.scalar.activation(st[:], src[:], AF.Sqrt)
                    nc.vector.reciprocal(dst[:], st[:])

                rstd0 = pc.tile([C, 1], F32, tag="rstd0")
                rsqrt(rstd0, var0, C, "r0")
                scale0 = pc.tile([C, 1], F32, tag="scale0")
                nc.vector.tensor_tensor(scale0[:], rstd0[:], g0c,
                                        op=ALU.mult)
                w1p = pc.tile([C, CM], BF16, tag="w1p")
                nc.vector.tensor_scalar(w1p[:], w1e[0:C, :], scale0[:], None,
                                        op0=ALU.mult)
                nshift0 = pc.tile([C, 1], F32, tag="nshift0")
                nc.vector.scalar_tensor_tensor(
                    nshift0[:], mom[:, 0:1], scale0[:], be0c,
                    op0=ALU.mult, op1=ALU.subtract)

                sh1e = pc.tile([C + 1, 1], F32, tag="sh1e")
                nc.vector.memset(sh1e[C:C + 1, :], -1.0)
                nc.vector.tensor_copy(out=sh1e[0:C, :], in_=nshift0[:])
                psb1 = psT.tile([CM, 1], F32, tag="misc", name="psb1")
                nc.tensor.matmul(psb1[:], lhsT=w1e, rhs=sh1e[:],
                                 start=True, stop=True)
                bias1 = pc.tile([CM, 1], F32, tag="bias1")
                nc.vector.tensor_scalar_mul(bias1[:], psb1[:], -1.0)

                nc.tensor.matmul(psoL[:], lhsT=w1p[:], rhs=ag_y[:, 0:512],
                                 start=False, stop=True)
                nc.tensor.matmul(psoR[:], lhsT=w1p[:], rhs=ag_y[:, 512:1024],
                                 start=False, stop=True)
                h_own = pb.tile([CM, 1024], BF16, tag="h_own")
                nc.scalar.activation(h_own[:, 0:512], psoL[:], AF.Prelu,
                                     bias=bias1[:], scale=1.0, alpha=SLOPE,
                                     accum_out=accB[0][:, 0:1])
                nc.scalar.activation(h_own[:, 512:1024], psoR[:], AF.Prelu,
                                     bias=bias1[:], scale=1.0, alpha=SLOPE,
                                     accum_out=accB[0][:, 4:5])
                hsq = pw.tile([CM, 1024], BF16, tag="hsq")
                nc.vector.scalar_tensor_tensor(
                    hsq[:, 0:512], h_own[:, 0:512], 1.0, h_own[:, 0:512],
                    op0=ALU.mult, op1=ALU.mult,
                    accum_out=accB[0][:, 1:2])
                nc.vector.scalar_tensor_tensor(
                    hsq[:, 512:1024], h_own[:, 512:1024], 1.0,
                    h_own[:, 512:1024],
                    op0=ALU.mult, op1=ALU.mult,
                    accum_out=accB[0][:, 5:6])
                nc.vector.tensor_tensor(accB[0][:, 0:2], accB[0][:, 0:2],
                                        accB[0][:, 4:6], op=ALU.add)

            # ---- butterfly #2 (BN1 stats): cores re-synchronized by the CC
            with tc.tile_critical(no_gpsimd_drain=True):
                butterfly(accB, rxB, rsemB, psemB, vsemB)

            with tc.tile_pool(name="psT2", bufs=1, space="PSUM") as psT2:
                yres = pb.tile([C, 1024], F32, tag="yres")
                nc.vector.tensor_scalar(yres[:], ag_y[:],
                                        scale0[:], nshift0[:],
                                        op0=ALU.mult, op1=ALU.subtract)
                nc.vector.tensor_tensor(yres[:], yres[:], wto,
                                        op=ALU.add)

                tot1 = accB[2][:, 0:2]
                mom1 = pc.tile([CM, 2], F32, tag="mom1")
                nc.vector.tensor_scalar_mul(mom1[:], tot1, 1.0 / NT)
                var1 = pc.tile([CM, 1], F32, tag="var1")
                nc.vector.tensor_tensor(var1[:], mom1[:, 0:1], mom1[:, 0:1],
                                        op=ALU.mult)
                nc.vector.scalar_tensor_tensor(
                    var1[:], mom1[:, 1:2], EPS, var1[:],
                    op0=ALU.add, op1=ALU.subtract)
                rstd1 = pc.tile([CM, 1], F32, tag="rstd1")
                rsqrt(rstd1, var1, CM, "r1")
                scale1 = pc.tile([CM, 1], F32, tag="scale1")
                nc.vector.tensor_tensor(scale1[:], rstd1[:], g1c,
                                        op=ALU.mult)
                nshift1 = pc.tile([CM, 1], F32, tag="nshift1")
                nc.vector.scalar_tensor_tensor(
                    nshift1[:], mom1[:, 0:1], scale1[:], be1c,
                    op0=ALU.mult, op1=ALU.subtract)
                w2p = pc.tile([CM, C], BF16, tag="w2p")
                nc.vector.tensor_scalar(w2p[:], w2, scale1[:], None,
                                        op0=ALU.mult)
                psb2 = psT2.tile([C, 1], F32, tag="misc", name="psb2")
                nc.tensor.matmul(psb2[:], lhsT=w2, rhs=nshift1[:],
                                 start=True, stop=True)
                bias2 = pc.tile([C, 1], F32, tag="bias2")
                nc.vector.tensor_tensor(bias2[:], b2c, psb2[:],
                                        op=ALU.subtract)

                out_sb = pw.tile([C, 1024], F32, tag="outsb")
                for c0 in (0, 512):
                    psd = psT2.tile([C, 512], F32, tag="misc",
                                    name=f"psd{c0}")
                    nc.tensor.matmul(psd[:], lhsT=w2p[:],
                                     rhs=h_own[:, c0:c0 + 512],
                                     start=True, stop=True)
                    nc.vector.scalar_tensor_tensor(
                        out_sb[:, c0:c0 + 512], psd[:], bias2[:],
                        yres[:, c0:c0 + 512], op0=ALU.add, op1=ALU.add)
                    nc.scalar.dma_start(out=out_d[:, c0:c0 + 512],
                                        in_=out_sb[:, c0:c0 + 512])

    nc.compile()
    return nc


_NC_CACHE = {}


def _get_module():
    if "nc" not in _NC_CACHE:
        _NC_CACHE["nc"] = _build_module()
    return _NC_CACHE["nc"]


BF = ml_dtypes.bfloat16


def _split_hi_lo(x):
    hi = x.astype(BF)
    lo = (x - hi.astype(np.float32)).astype(BF)
    return hi, lo


def _host_prep(inputs):
    pos = np.asarray(inputs["positions"], np.float32)
    w = np.asarray(inputs["weights"], np.float32)
    kp = np.asarray(inputs["kernel_pos"], np.float32)
    cw = np.asarray(inputs["conv_w"], np.float32)
    posb = pos.reshape(NB, N, 2)
    wb = w.reshape(NB, N, C)
    kk2 = 0.5 * (kp ** 2).sum(1)
    cwall = cw.transpose(1, 0, 2).reshape(C, KC).astype(BF)
    wTfull = np.ascontiguousarray(w.T)
    w1 = np.asarray(inputs["w1"], np.float32)
    w2 = np.asarray(inputs["w2"], np.float32)
    w1e = np.concatenate(
        [w1, np.asarray(inputs["b1"], np.float32).reshape(1, CM)], axis=0)
    p128 = np.zeros((128, 44), np.float32)
    p128[:, 8:40] = w2
    p128[:, 40] = np.asarray(inputs["bn1_gamma"], np.float32)
    p128[:, 41] = np.asarray(inputs["bn1_beta"], np.float32)
    p128[:, 42] = np.int32(1).view(np.float32)

    in_maps = []
    for j in range(NCORES):
        b = j
        p = posb[b]
        pi = p
        xh, xl = _split_hi_lo(p[:, 0])
        yh, yl = _split_hi_lo(p[:, 1])
        one = np.ones(N, BF)
        pn8 = np.stack([xh, xh, xl, yh, yh, yl, one, one])
        bias = -0.5 * (pi ** 2).sum(1)
        bh, bl = _split_hi_lo(bias)
        xih, xil = _split_hi_lo(pi[:, 0])
        yih, yil = _split_hi_lo(pi[:, 1])
        pi8 = np.stack([xih, xil, xih, yih, yil, yih, bh, bl])
        packp = np.concatenate([pn8, pi8], axis=1)

        packw = np.empty((C, 2464), BF)
        packw[:, 0:1024] = wb[b].T.astype(BF)
        packw[:, 1024:2048] = wTfull[:, j * 1024:(j + 1) * 1024].astype(BF)
        packw[:, 2048:2176] = w1.astype(BF)
        packw[:, 2176:2464] = cwall

        p128j = p128.copy()
        p128j[:, 0:8] = (-0.5 * (p ** 2).sum(1)).reshape(8, 128).T
        dotn = (p @ kp.T).astype(np.float32)
        Bmat = np.exp(-dotn - kk2[None, :]).astype(np.float32)
        Amat = np.exp((pi @ kp.T).astype(np.float32)).astype(np.float32)
        packf = np.empty((128, 4608), BF)
        packf[:, 0:8 * KC] = np.broadcast_to(
            Bmat.reshape(8, 128, K, 1).transpose(1, 0, 2, 3),
            (128, 8, K, C)).reshape(128, 8 * KC).astype(BF)
        packf[:, 8 * KC:] = np.broadcast_to(
            Amat.reshape(8, 128, K, 1).transpose(1, 0, 2, 3),
            (128, 8, K, C)).reshape(128, 8 * KC).astype(BF)

        packs = np.zeros((C + 1, 1156), np.float32)
        packs[:, 0:CM] = w1e
        packs[0:C, 128] = np.asarray(inputs["bn_gamma"], np.float32)
        packs[0:C, 129] = np.asarray(inputs["bn_beta"], np.float32)
        packs[0:C, 130] = np.asarray(inputs["b2"], np.float32)
        packs[0:C, 131:1155] = wTfull[:, j * 1024:(j + 1) * 1024]

        in_maps.append(dict(
            packp=np.ascontiguousarray(packp),
            packw=np.ascontiguousarray(packw),
            packf=np.ascontiguousarray(packf),
            packs=np.ascontiguousarray(packs),
            p128=np.ascontiguousarray(p128j)))
    return in_maps


def _run(inputs, trace=False):
    nc = _get_module()
    in_maps = _host_prep(inputs)
    res = run_bass_kernel_spmd(nc, in_maps, core_ids=list(range(NCORES)),
                               trace=trace)
    out = np.concatenate([np.asarray(res.results[j]["out"])
                          for j in range(NCORES)], axis=1)
    return np.ascontiguousarray(out.T), res


def kernel(**inputs):
    out, _ = _run(inputs, trace=False)
    return out
